# revision 17
# baseline (speedup 1.0000x reference)
"""GCNConv kernel for Trainium2, SPMD over 8 NeuronCores.

Math (matches the reference):
    row = [edge_index[0], arange(N)]; col = [edge_index[1], arange(N)]
    deg = bincount(row); dis = deg ** -0.5
    agg[c] = sum_{e: col_e == c} dis[row_e] * dis[c] * x[row_e]
    out = agg @ W.T + b

Distribution: edges are sorted by destination column; core k owns
destination columns [k*12500, (k+1)*12500).  Outputs are disjoint across
cores -> no collectives.

Host preprocessing builds, per core, the edge-expanded message stream
x_dis[row_e] (bf16, already scaled by the source-side dis factor) laid out
in [128-edge x 128-feat] tiles grouped by destination column block, plus an
exact fp8 0/1 one-hot S mapping each edge slot to its local destination
column.  (The per-edge expansion is done host-side: the batched device
gather primitive -- InstDMAGatherAnt -- corrupts index values >= 256 on
this toolchain (empirically the idx stream is rounded like an 8-bit-
mantissa float), and the working [128,1]-offset indirect-DMA fallback
costs a measured 1.44 us per 128 edges = ~2.7 ms/core, 8x over the memory
roofline.  Multi-offset indirect DMA ([128,K>=2]) gathers wrong rows for
partitions >= 64 -- all verified on hardware.)

Per-core device program (the scatter/segment-sum and the linear):
  - for each group of GRP=4 column blocks (128 cols each):
      * stream the group's message tiles into SBUF (contiguous DMA)
      * per block: K matmuls  psum[d, c] += G_tile^T @ S_tile  accumulate
        the block's transposed segment sums in PSUM
      * drain PSUM with a fused multiply by dis[col] (fp32, broadcast
        tile), fp32 matmul with W^T, per-partition bias add, store the
        [dout, col] tile to the transposed output
  - host transposes each core's [128, 12500] slab into the final output.
"""

import numpy as np
import ml_dtypes

BF16 = ml_dtypes.bfloat16
FP8 = ml_dtypes.float8_e4m3

N_NODES = 100000
D = 128
N_CORES = 8
BLK = 128
GRP = 4  # column blocks per DMA group


def _preprocess(x, edge_index, W, b, n_cores=N_CORES, grp=GRP):
    """Host-side index preprocessing and input sharding."""
    n = x.shape[0]
    d = x.shape[1]
    assert n % n_cores == 0
    cpc = n // n_cores  # destination columns per core
    nblk = -(-cpc // BLK)  # column blocks per core
    pcols = nblk * BLK
    ng = -(-nblk // grp)  # groups per core

    idt = edge_index.dtype
    loop = np.arange(n, dtype=idt)
    row = np.concatenate([np.asarray(edge_index[0]), loop])
    col = np.concatenate([np.asarray(edge_index[1]), loop])

    deg = np.bincount(row, minlength=n)
    dis = (deg.astype(np.float64) ** -0.5).astype(np.float32)

    x_dis = (np.asarray(x) * dis[:, None]).astype(BF16)

    order = np.argsort(col, kind="stable")
    rs = row[order].astype(np.int64)
    cs = col[order].astype(np.int64)

    core = cs // cpc
    lb = (cs - core * cpc) // BLK  # block within core
    cloc = (cs - core * cpc) % BLK  # column within block

    key = core * nblk + lb
    counts = np.bincount(key, minlength=n_cores * nblk)
    starts = np.concatenate([[0], np.cumsum(counts)[:-1]])
    rank = np.arange(len(cs)) - starts[key]
    kmax = int(-(-counts.max() // 128))

    g = lb // grp
    bl = lb % grp
    kt = rank // 128
    p = rank % 128
    w_idx = grp * kmax

    # edge-expanded message stream: [core][g][p][bl*kmax+kt][128 feat]
    xe = np.zeros((n_cores, ng, 128, w_idx, d), BF16)
    flat_tile = ((core * ng + g) * 128 + p) * w_idx + bl * kmax + kt
    xe.reshape(-1, d)[flat_tile] = x_dis[rs]

    # per-slot local destination column (bf16 holds 0..127 exactly); pads get
    # 255 which never matches the device-side iota compare -> zero one-hot row
    clocs = np.full((n_cores, ng, 128, w_idx), 255.0, BF16)
    flat_c = ((core * ng + g) * 128 + p) * w_idx + bl * kmax + kt
    clocs.reshape(-1)[flat_c] = cloc.astype(BF16)
    iota = np.broadcast_to(np.arange(BLK, dtype=np.float32), (128, BLK)).astype(BF16)

    disb_all = np.zeros((n_cores, 128, pcols), np.float32)
    for k in range(n_cores):
        dc = np.zeros(pcols, np.float32)
        dc[:cpc] = dis[k * cpc : (k + 1) * cpc]
        disb_all[k] = dc[None, :]

    wt = np.ascontiguousarray(np.asarray(W).T.astype(np.float32))
    bias = np.asarray(b).astype(np.float32).reshape(d, 1)

    in_maps = []
    for k in range(n_cores):
        in_maps.append(
            {
                "xe": xe[k].reshape(ng, 128, w_idx * d),
                "clocs": clocs[k],
                "iota": iota,
                "wt": wt,
                "bias": bias,
                "disb": disb_all[k],
            }
        )
    meta = dict(n=n, d=d, cpc=cpc, nblk=nblk, pcols=pcols, ng=ng, kmax=kmax, grp=grp)
    return in_maps, meta


def _build_program(meta):
    import concourse.bacc as bacc
    import concourse.tile as tile
    from concourse import mybir

    d = meta["d"]
    ng = meta["ng"]
    grp = meta["grp"]
    kmax = meta["kmax"]
    pcols = meta["pcols"]
    nblk = meta["nblk"]
    w_idx = grp * kmax

    f32 = mybir.dt.float32
    bf16 = mybir.dt.bfloat16
    fp8 = mybir.dt.float8e4

    nc = bacc.Bacc("TRN2", target_bir_lowering=False, debug=False)
    xe_t = nc.declare_dram_parameter("xe", [ng, 128, w_idx * d], bf16, isOutput=False)
    c_t = nc.declare_dram_parameter("clocs", [ng, 128, w_idx], bf16, isOutput=False)
    io_t = nc.declare_dram_parameter("iota", [128, BLK], bf16, isOutput=False)
    wt_t = nc.declare_dram_parameter("wt", [d, d], f32, isOutput=False)
    b_t = nc.declare_dram_parameter("bias", [d, 1], f32, isOutput=False)
    d_t = nc.declare_dram_parameter("disb", [128, pcols], f32, isOutput=False)
    o_t = nc.declare_dram_parameter("outT", [128, pcols], f32, isOutput=True)

    with tile.TileContext(nc) as tc:
        with (
            tc.tile_pool(name="const", bufs=1) as constp,
            tc.tile_pool(name="gather", bufs=3) as gatherp,
            tc.tile_pool(name="sohp", bufs=3) as sohp,
            tc.tile_pool(name="clocp", bufs=3) as clocp,
            tc.tile_pool(name="aggp", bufs=3) as aggp,
            tc.tile_pool(name="outp", bufs=2) as outp,
            tc.tile_pool(name="psA", bufs=4, space="PSUM") as psa,
            tc.tile_pool(name="psB", bufs=2, space="PSUM") as psb,
        ):
            # constants go on the scalar (ACT) HWDGE queue so the sync queue
            # stays a pure xe/soh prefetch stream
            wt_sb = constp.tile([d, d], f32)
            nc.scalar.dma_start(out=wt_sb[:], in_=wt_t[:])
            b_sb = constp.tile([d, 1], f32)
            nc.scalar.dma_start(out=b_sb[:], in_=b_t[:])
            iota_sb = constp.tile([128, BLK], bf16)
            nc.scalar.dma_start(out=iota_sb[:], in_=io_t[:])
            disb_sb = constp.tile([128, pcols], f32)
            nc.scalar.dma_start(out=disb_sb[:], in_=d_t[:])

            for gi in range(ng):
                ar = gatherp.tile([128, w_idx * d], bf16)
                nc.sync.dma_start(out=ar[:], in_=xe_t[gi])
                cloc_sb = clocp.tile([128, w_idx], bf16)
                nc.sync.dma_start(out=cloc_sb[:], in_=c_t[gi])
                # build the fp8 0/1 one-hot on-device: S[p, t, c] = (cloc[p,t] == c)
                # split per block: DVE free-dim per op stays <= 8192
                s_sb = sohp.tile([128, w_idx * 128], fp8)
                for bl0 in range(grp):
                    t0, t1 = bl0 * kmax, (bl0 + 1) * kmax
                    nc.vector.tensor_tensor(
                        out=s_sb[:, t0 * 128 : t1 * 128].rearrange(
                            "p (t c) -> p t c", c=128
                        ),
                        in0=iota_sb[:]
                        .rearrange("p (a c) -> p a c", a=1)
                        .broadcast_to([128, kmax, 128]),
                        in1=cloc_sb[:, t0:t1]
                        .rearrange("p (t a) -> p t a", a=1)
                        .broadcast_to([128, kmax, 128]),
                        op=mybir.AluOpType.is_equal,
                    )

                gblk = min(grp, nblk - gi * grp)  # blocks in this group
                out_g = outp.tile([128, grp * 128], f32, tag="outg")
                for bl in range(gblk):
                    blk = gi * grp + bl
                    ps = psa.tile([128, 128], f32)
                    for kt in range(kmax):
                        off = (bl * kmax + kt) * 128
                        nc.tensor.matmul(
                            out=ps[:],
                            lhsT=ar[:, off : off + 128],
                            rhs=s_sb[:, off : off + 128],
                            start=(kt == 0),
                            stop=(kt == kmax - 1),
                        )
                    agg_sb = aggp.tile([128, 128], f32)
                    nc.vector.tensor_tensor(
                        out=agg_sb[:],
                        in0=ps[:],
                        in1=disb_sb[:, blk * 128 : (blk + 1) * 128],
                        op=mybir.AluOpType.mult,
                    )
                    ps2 = psb.tile([128, 128], f32)
                    nc.tensor.matmul(
                        out=ps2[:], lhsT=wt_sb[:], rhs=agg_sb[:], start=True, stop=True
                    )
                    nc.vector.tensor_scalar_add(
                        out=out_g[:, bl * 128 : (bl + 1) * 128],
                        in0=ps2[:],
                        scalar1=b_sb[:],
                    )
                # one batched store per group, on the scalar HWDGE queue
                nc.scalar.dma_start(
                    out=o_t[:, gi * grp * 128 : gi * grp * 128 + gblk * 128],
                    in_=out_g[:, : gblk * 128],
                )
    nc.compile()
    return nc


def _run(in_maps, meta, trace=False):
    from concourse.bass_utils import run_bass_kernel_spmd

    nc = _build_program(meta)
    n_cores = len(in_maps)
    res = run_bass_kernel_spmd(nc, in_maps, list(range(n_cores)), trace=trace)
    return res


def _assemble(results, meta, n_cores):
    cpc = meta["cpc"]
    out = np.empty((meta["n"], meta["d"]), np.float32)
    for k in range(n_cores):
        out[k * cpc : (k + 1) * cpc, :] = results[k]["outT"][:, :cpc].T
    return out


def kernel(x, edge_index, W, b):
    in_maps, meta = _preprocess(x, edge_index, W, b)
    res = _run(in_maps, meta, trace=False)
    return _assemble(res.results, meta, N_CORES)


# revision 18
# speedup vs baseline: 1.1636x; 1.1636x over previous
"""GCNConv kernel for Trainium2, SPMD over 8 NeuronCores.

Math (matches the reference):
    row = [edge_index[0], arange(N)]; col = [edge_index[1], arange(N)]
    deg = bincount(row); dis = deg ** -0.5
    agg[c] = sum_{e: col_e == c} dis[row_e] * dis[c] * x[row_e]
    out = agg @ W.T + b

Distribution: edges are sorted by destination column; core k owns
destination columns [k*12500, (k+1)*12500).  Outputs are disjoint across
cores -> no collectives.

Host preprocessing builds, per core, the edge-expanded message stream
x_dis[row_e] (bf16, pre-scaled by the source-side dis factor) laid out in
[128-edge x 128-feat] tiles grouped by destination column block, plus an
exact fp8 0/1 one-hot S mapping each edge slot to its local destination
column.  (The per-edge expansion is done host-side: the batched device
gather primitive -- InstDMAGatherAnt -- corrupts index values >= 256 on
this toolchain (the idx stream is rounded like an 8-bit-mantissa float,
verified empirically), multi-offset indirect DMA ([128,K>=2] offsets)
gathers wrong rows for partitions >= 64, and the working [128,1]-offset
indirect-DMA fallback costs a measured 1.44 us per 128 edges = ~2.7 ms
per core, 8x over the memory roofline.  All verified on hardware.)

Per-core device program (the scatter/segment-sum and the linear):
  - for each group of GRP=4 column blocks (128 destination columns each):
      * stream the group's message tiles + one-hot tiles into SBUF
        (contiguous DMAs on the sync HWDGE queue, triple-buffered)
      * per block b: kb[b] matmuls  psum[d, c] += G_tile^T @ S_tile
        accumulate the block's transposed segment sums in PSUM
      * drain PSUM with a fused multiply by dis[col] (fp32 broadcast
        tile, DVE), fp32 matmul with W^T, per-partition bias add
      * one batched output store per group on the scalar HWDGE queue
        (keeps the sync queue a pure prefetch stream)
  - host transposes each core's [128, 12500] slab into the final output.

Per-block tile counts kb[b] (max over cores) instead of a global max
trim ~6% of the streamed bytes and matmuls.
"""

import numpy as np
import ml_dtypes

BF16 = ml_dtypes.bfloat16
FP8 = ml_dtypes.float8_e4m3

N_NODES = 100000
D = 128
N_CORES = 8
BLK = 128
GRP = 4  # column blocks per DMA group


def _preprocess(x, edge_index, W, b, n_cores=N_CORES, grp=GRP):
    """Host-side index preprocessing and input sharding."""
    n = x.shape[0]
    d = x.shape[1]
    assert n % n_cores == 0
    cpc = n // n_cores  # destination columns per core
    nblk = -(-cpc // BLK)  # column blocks per core
    pcols = nblk * BLK
    ng = -(-nblk // grp)  # groups per core

    idt = edge_index.dtype
    loop = np.arange(n, dtype=idt)
    row = np.concatenate([np.asarray(edge_index[0]), loop])
    col = np.concatenate([np.asarray(edge_index[1]), loop])

    deg = np.bincount(row, minlength=n)
    dis = (deg.astype(np.float64) ** -0.5).astype(np.float32)

    x_dis = (np.asarray(x) * dis[:, None]).astype(BF16)

    order = np.argsort(col, kind="stable")
    rs = row[order].astype(np.int64)
    cs = col[order].astype(np.int64)

    core = cs // cpc
    lb = (cs - core * cpc) // BLK  # block within core
    cloc = (cs - core * cpc) % BLK  # column within block

    key = core * nblk + lb
    counts = np.bincount(key, minlength=n_cores * nblk).reshape(n_cores, nblk)
    starts = np.concatenate([[0], np.cumsum(counts.reshape(-1))[:-1]])
    rank = np.arange(len(cs)) - starts[key]

    # per-block tile count = max over cores; toff = cumulative tile offsets
    kb = -(-counts.max(axis=0) // 128)  # [nblk]
    toff = np.concatenate([[0], np.cumsum(kb)]).astype(np.int64)  # [nblk+1]
    t_total = int(toff[-1])

    tglob = toff[lb] + rank // 128
    p = rank % 128

    # edge-expanded message stream: [core][p][tile][feat]
    xe = np.zeros((n_cores, 128, t_total, d), BF16)
    flat_tile = (core * 128 + p) * t_total + tglob
    xe.reshape(-1, d)[flat_tile] = x_dis[rs]

    s_all = np.zeros((n_cores, 128, t_total * 128), FP8)
    flat_s = ((core * 128 + p) * t_total + tglob) * 128 + cloc
    s_all.reshape(-1)[flat_s] = FP8(1.0)

    disb_all = np.zeros((n_cores, 128, pcols), np.float32)
    for k in range(n_cores):
        dc = np.zeros(pcols, np.float32)
        dc[:cpc] = dis[k * cpc : (k + 1) * cpc]
        disb_all[k] = dc[None, :]

    wt = np.ascontiguousarray(np.asarray(W).T.astype(np.float32))
    bias = np.asarray(b).astype(np.float32).reshape(d, 1)

    in_maps = []
    for k in range(n_cores):
        in_maps.append(
            {
                "xe": xe[k].reshape(128, t_total * d),
                "soh": s_all[k],
                "wt": wt,
                "bias": bias,
                "disb": disb_all[k],
            }
        )
    meta = dict(
        n=n,
        d=d,
        cpc=cpc,
        nblk=nblk,
        pcols=pcols,
        ng=ng,
        grp=grp,
        kb=[int(v) for v in kb],
        toff=[int(v) for v in toff],
        t_total=t_total,
    )
    return in_maps, meta


def _build_program(meta):
    import concourse.bacc as bacc
    import concourse.tile as tile
    from concourse import mybir

    d = meta["d"]
    ng = meta["ng"]
    grp = meta["grp"]
    pcols = meta["pcols"]
    nblk = meta["nblk"]
    kb = meta["kb"]
    toff = meta["toff"]
    t_total = meta["t_total"]

    f32 = mybir.dt.float32
    bf16 = mybir.dt.bfloat16
    fp8 = mybir.dt.float8e4

    # max tiles in any group (sizes the SBUF pools)
    gt_max = max(
        toff[min((gi + 1) * grp, nblk)] - toff[gi * grp] for gi in range(ng)
    )

    nc = bacc.Bacc("TRN2", target_bir_lowering=False, debug=False)
    xe_t = nc.declare_dram_parameter("xe", [128, t_total * d], bf16, isOutput=False)
    s_t = nc.declare_dram_parameter("soh", [128, t_total * 128], fp8, isOutput=False)
    wt_t = nc.declare_dram_parameter("wt", [d, d], f32, isOutput=False)
    b_t = nc.declare_dram_parameter("bias", [d, 1], f32, isOutput=False)
    d_t = nc.declare_dram_parameter("disb", [128, pcols], f32, isOutput=False)
    o_t = nc.declare_dram_parameter("outT", [128, pcols], f32, isOutput=True)

    with tile.TileContext(nc) as tc:
        with (
            tc.tile_pool(name="const", bufs=1) as constp,
            tc.tile_pool(name="gather", bufs=3) as gatherp,
            tc.tile_pool(name="sohp", bufs=3) as sohp,
            tc.tile_pool(name="aggp", bufs=3) as aggp,
            tc.tile_pool(name="outp", bufs=2) as outp,
            tc.tile_pool(name="psA", bufs=4, space="PSUM") as psa,
            tc.tile_pool(name="psB", bufs=2, space="PSUM") as psb,
        ):
            # constants go on the scalar (ACT) HWDGE queue so the sync queue
            # stays a pure xe/soh prefetch stream
            wt_sb = constp.tile([d, d], f32)
            nc.scalar.dma_start(out=wt_sb[:], in_=wt_t[:])
            b_sb = constp.tile([d, 1], f32)
            nc.scalar.dma_start(out=b_sb[:], in_=b_t[:])
            disb_sb = constp.tile([128, pcols], f32)
            nc.scalar.dma_start(out=disb_sb[:], in_=d_t[:])

            for gi in range(ng):
                b0 = gi * grp
                gblk = min(grp, nblk - b0)
                g_t0, g_t1 = toff[b0], toff[b0 + gblk]
                gt = g_t1 - g_t0

                ar = gatherp.tile([128, gt_max * d], bf16, tag="ar")
                nc.sync.dma_start(
                    out=ar[:, : gt * d], in_=xe_t[:, g_t0 * d : g_t1 * d]
                )
                s_sb = sohp.tile([128, gt_max * 128], fp8, tag="soh")
                nc.sync.dma_start(
                    out=s_sb[:, : gt * 128], in_=s_t[:, g_t0 * 128 : g_t1 * 128]
                )

                out_g = outp.tile([128, grp * 128], f32, tag="outg")
                for bl in range(gblk):
                    blk = b0 + bl
                    loc0 = toff[blk] - g_t0
                    ps = psa.tile([128, 128], f32)
                    for kt in range(kb[blk]):
                        off = (loc0 + kt) * 128
                        nc.tensor.matmul(
                            out=ps[:],
                            lhsT=ar[:, off : off + 128],
                            rhs=s_sb[:, off : off + 128],
                            start=(kt == 0),
                            stop=(kt == kb[blk] - 1),
                        )
                    agg_sb = aggp.tile([128, 128], f32)
                    nc.vector.tensor_tensor(
                        out=agg_sb[:],
                        in0=ps[:],
                        in1=disb_sb[:, blk * 128 : (blk + 1) * 128],
                        op=mybir.AluOpType.mult,
                    )
                    ps2 = psb.tile([128, 128], f32)
                    nc.tensor.matmul(
                        out=ps2[:], lhsT=wt_sb[:], rhs=agg_sb[:], start=True, stop=True
                    )
                    nc.vector.tensor_scalar_add(
                        out=out_g[:, bl * 128 : (bl + 1) * 128],
                        in0=ps2[:],
                        scalar1=b_sb[:],
                    )
                nc.scalar.dma_start(
                    out=o_t[:, b0 * 128 : b0 * 128 + gblk * 128],
                    in_=out_g[:, : gblk * 128],
                )
    nc.compile()
    return nc


def _run(in_maps, meta, trace=False):
    from concourse.bass_utils import run_bass_kernel_spmd

    nc = _build_program(meta)
    n_cores = len(in_maps)
    res = run_bass_kernel_spmd(nc, in_maps, list(range(n_cores)), trace=trace)
    return res


def _assemble(results, meta, n_cores):
    cpc = meta["cpc"]
    out = np.empty((meta["n"], meta["d"]), np.float32)
    for k in range(n_cores):
        out[k * cpc : (k + 1) * cpc, :] = results[k]["outT"][:, :cpc].T
    return out


def kernel(x, edge_index, W, b):
    in_maps, meta = _preprocess(x, edge_index, W, b)
    res = _run(in_maps, meta, trace=False)
    return _assemble(res.results, meta, N_CORES)


# revision 21
# speedup vs baseline: 1.1877x; 1.0207x over previous
"""GCNConv kernel for Trainium2, SPMD over 8 NeuronCores.

Math (matches the reference):
    row = [edge_index[0], arange(N)]; col = [edge_index[1], arange(N)]
    deg = bincount(row); dis = deg ** -0.5
    agg[c] = sum_{e: col_e == c} dis[row_e] * dis[c] * x[row_e]
    out = agg @ W.T + b

Distribution: edges are sorted by destination column; core k owns
destination columns [k*12500, (k+1)*12500).  Outputs are disjoint across
cores -> no collectives.

Host preprocessing builds, per core, the edge-expanded message stream
x_dis[row_e] (bf16, pre-scaled by the source-side dis factor) laid out in
[128-edge x 128-feat] tiles grouped by destination column block, plus an
exact fp8 0/1 one-hot S mapping each edge slot to its local destination
column.  (The per-edge expansion is done host-side: the batched device
gather primitive -- InstDMAGatherAnt -- corrupts index values >= 256 on
this toolchain (the idx stream is rounded like an 8-bit-mantissa float,
verified empirically), multi-offset indirect DMA ([128,K>=2] offsets)
gathers wrong rows for partitions >= 64, and the working [128,1]-offset
indirect-DMA fallback costs a measured 1.44 us per 128 edges = ~2.7 ms
per core, 8x over the memory roofline.  All verified on hardware.)

Per-core device program (the scatter/segment-sum and the linear):
  - for each group of GRP=4 column blocks (128 destination columns each):
      * stream the group's message tiles + one-hot tiles into SBUF
        (contiguous DMAs on the sync HWDGE queue, triple-buffered)
      * per block b: kb[b] matmuls  psum[d, c] += G_tile^T @ S_tile
        accumulate the block's transposed segment sums in PSUM
      * drain PSUM with a fused multiply by dis[col] (fp32 broadcast
        tile, DVE), fp32 matmul with W^T, per-partition bias add
      * one batched output store per group on the scalar HWDGE queue
        (keeps the sync queue a pure prefetch stream)
  - host transposes each core's [128, 12500] slab into the final output.

Per-block tile counts kb[b] (max over cores) instead of a global max
trim ~6% of the streamed bytes and matmuls.
"""

import numpy as np
import ml_dtypes

BF16 = ml_dtypes.bfloat16
FP8 = ml_dtypes.float8_e4m3

N_NODES = 100000
D = 128
N_CORES = 8
BLK = 128
GRP = 4  # column blocks per DMA group


def _preprocess(x, edge_index, W, b, n_cores=N_CORES, grp=GRP):
    """Host-side index preprocessing and input sharding."""
    n = x.shape[0]
    d = x.shape[1]
    assert n % n_cores == 0
    cpc = n // n_cores  # destination columns per core
    nblk = -(-cpc // BLK)  # column blocks per core
    pcols = nblk * BLK
    ng = -(-nblk // grp)  # groups per core

    idt = edge_index.dtype
    loop = np.arange(n, dtype=idt)
    row = np.concatenate([np.asarray(edge_index[0]), loop])
    col = np.concatenate([np.asarray(edge_index[1]), loop])

    deg = np.bincount(row, minlength=n)
    dis = (deg.astype(np.float64) ** -0.5).astype(np.float32)

    x_dis = (np.asarray(x) * dis[:, None]).astype(BF16)

    order = np.argsort(col, kind="stable")
    rs = row[order].astype(np.int64)
    cs = col[order].astype(np.int64)

    core = cs // cpc
    lb = (cs - core * cpc) // BLK  # block within core
    cloc = (cs - core * cpc) % BLK  # column within block

    key = core * nblk + lb
    counts = np.bincount(key, minlength=n_cores * nblk).reshape(n_cores, nblk)
    starts = np.concatenate([[0], np.cumsum(counts.reshape(-1))[:-1]])
    rank = np.arange(len(cs)) - starts[key]

    # per-block tile count = max over cores; toff = cumulative tile offsets
    kb = -(-counts.max(axis=0) // 128)  # [nblk]
    toff = np.concatenate([[0], np.cumsum(kb)]).astype(np.int64)  # [nblk+1]
    t_total = int(toff[-1])

    tglob = toff[lb] + rank // 128
    p = rank % 128

    # edge-expanded message stream: [core][p][tile][feat]
    xe = np.zeros((n_cores, 128, t_total, d), BF16)
    flat_tile = (core * 128 + p) * t_total + tglob
    xe.reshape(-1, d)[flat_tile] = x_dis[rs]

    s_all = np.zeros((n_cores, 128, t_total * 128), FP8)
    flat_s = ((core * 128 + p) * t_total + tglob) * 128 + cloc
    s_all.reshape(-1)[flat_s] = FP8(1.0)

    disb_all = np.zeros((n_cores, 128, pcols), np.float32)
    for k in range(n_cores):
        dc = np.zeros(pcols, np.float32)
        dc[:cpc] = dis[k * cpc : (k + 1) * cpc]
        disb_all[k] = dc[None, :]

    wt = np.ascontiguousarray(np.asarray(W).T.astype(np.float32))
    bias = np.asarray(b).astype(np.float32).reshape(d, 1)

    # group-major flat layout: each group's [128, gt*d] region is contiguous
    # in DRAM so its load DMA is one spray-able block
    xe4 = xe  # [cores, 128, t_total, d]
    s4 = s_all.reshape(n_cores, 128, t_total, 128)
    xe_parts, s_parts = [], []
    for gi in range(ng):
        b0 = gi * grp
        b1 = min(b0 + grp, nblk)
        t0, t1 = int(toff[b0]), int(toff[b1])
        xe_parts.append(xe4[:, :, t0:t1, :].reshape(n_cores, -1))
        s_parts.append(s4[:, :, t0:t1, :].reshape(n_cores, -1))
    xe_flat = np.ascontiguousarray(np.concatenate(xe_parts, axis=1))
    s_flat = np.ascontiguousarray(np.concatenate(s_parts, axis=1))

    in_maps = []
    for k in range(n_cores):
        in_maps.append(
            {
                "xe": xe_flat[k],
                "soh": s_flat[k],
                "wt": wt,
                "bias": bias,
                "disb": disb_all[k],
            }
        )
    meta = dict(
        n=n,
        d=d,
        cpc=cpc,
        nblk=nblk,
        pcols=pcols,
        ng=ng,
        grp=grp,
        kb=[int(v) for v in kb],
        toff=[int(v) for v in toff],
        t_total=t_total,
    )
    return in_maps, meta


def _build_program(meta):
    import concourse.bacc as bacc
    import concourse.tile as tile
    from concourse import mybir

    d = meta["d"]
    ng = meta["ng"]
    grp = meta["grp"]
    pcols = meta["pcols"]
    nblk = meta["nblk"]
    kb = meta["kb"]
    toff = meta["toff"]
    t_total = meta["t_total"]

    f32 = mybir.dt.float32
    bf16 = mybir.dt.bfloat16
    fp8 = mybir.dt.float8e4

    # max tiles in any group (sizes the SBUF pools)
    gt_max = max(
        toff[min((gi + 1) * grp, nblk)] - toff[gi * grp] for gi in range(ng)
    )

    nc = bacc.Bacc("TRN2", target_bir_lowering=False, debug=False)
    xe_t = nc.declare_dram_parameter("xe", [128 * t_total * d], bf16, isOutput=False)
    s_t = nc.declare_dram_parameter("soh", [128 * t_total * 128], fp8, isOutput=False)
    wt_t = nc.declare_dram_parameter("wt", [d, d], f32, isOutput=False)
    b_t = nc.declare_dram_parameter("bias", [d, 1], f32, isOutput=False)
    d_t = nc.declare_dram_parameter("disb", [128, pcols], f32, isOutput=False)
    o_t = nc.declare_dram_parameter("outT", [128, pcols], f32, isOutput=True)

    with tile.TileContext(nc) as tc:
        with (
            tc.tile_pool(name="const", bufs=1) as constp,
            tc.tile_pool(name="gather", bufs=3) as gatherp,
            tc.tile_pool(name="sohp", bufs=3) as sohp,
            tc.tile_pool(name="aggp", bufs=3) as aggp,
            tc.tile_pool(name="outp", bufs=2) as outp,
            tc.tile_pool(name="psA", bufs=4, space="PSUM") as psa,
            tc.tile_pool(name="psB", bufs=2, space="PSUM") as psb,
        ):
            # constants go on the scalar (ACT) HWDGE queue so the sync queue
            # stays a pure xe/soh prefetch stream
            wt_sb = constp.tile([d, d], f32)
            nc.scalar.dma_start(out=wt_sb[:], in_=wt_t[:])
            b_sb = constp.tile([d, 1], f32)
            nc.scalar.dma_start(out=b_sb[:], in_=b_t[:])
            disb_sb = constp.tile([128, pcols], f32)
            nc.scalar.dma_start(out=disb_sb[:], in_=d_t[:])

            for gi in range(ng):
                b0 = gi * grp
                gblk = min(grp, nblk - b0)
                g_t0, g_t1 = toff[b0], toff[b0 + gblk]
                gt = g_t1 - g_t0

                ar = gatherp.tile([128, gt_max * d], bf16, tag="ar")
                nc.sync.dma_start(
                    out=ar[:, : gt * d],
                    in_=xe_t[128 * g_t0 * d : 128 * g_t1 * d].rearrange(
                        "(p w) -> p w", p=128
                    ),
                )
                s_sb = sohp.tile([128, gt_max * 128], fp8, tag="soh")
                nc.sync.dma_start(
                    out=s_sb[:, : gt * 128],
                    in_=s_t[128 * g_t0 * 128 : 128 * g_t1 * 128].rearrange(
                        "(p w) -> p w", p=128
                    ),
                )

                out_g = outp.tile([128, grp * 128], f32, tag="outg")
                for bl in range(gblk):
                    blk = b0 + bl
                    loc0 = toff[blk] - g_t0
                    ps = psa.tile([128, 128], f32)
                    for kt in range(kb[blk]):
                        off = (loc0 + kt) * 128
                        nc.tensor.matmul(
                            out=ps[:],
                            lhsT=ar[:, off : off + 128],
                            rhs=s_sb[:, off : off + 128],
                            start=(kt == 0),
                            stop=(kt == kb[blk] - 1),
                        )
                    agg_sb = aggp.tile([128, 128], f32)
                    nc.vector.tensor_tensor(
                        out=agg_sb[:],
                        in0=ps[:],
                        in1=disb_sb[:, blk * 128 : (blk + 1) * 128],
                        op=mybir.AluOpType.mult,
                    )
                    ps2 = psb.tile([128, 128], f32)
                    nc.tensor.matmul(
                        out=ps2[:], lhsT=wt_sb[:], rhs=agg_sb[:], start=True, stop=True
                    )
                    nc.vector.tensor_scalar_add(
                        out=out_g[:, bl * 128 : (bl + 1) * 128],
                        in0=ps2[:],
                        scalar1=b_sb[:],
                    )
                nc.scalar.dma_start(
                    out=o_t[:, b0 * 128 : b0 * 128 + gblk * 128],
                    in_=out_g[:, : gblk * 128],
                )
    nc.compile()
    return nc


def _run(in_maps, meta, trace=False):
    from concourse.bass_utils import run_bass_kernel_spmd

    nc = _build_program(meta)
    n_cores = len(in_maps)
    res = run_bass_kernel_spmd(nc, in_maps, list(range(n_cores)), trace=trace)
    return res


def _assemble(results, meta, n_cores):
    cpc = meta["cpc"]
    out = np.empty((meta["n"], meta["d"]), np.float32)
    for k in range(n_cores):
        out[k * cpc : (k + 1) * cpc, :] = results[k]["outT"][:, :cpc].T
    return out


def kernel(x, edge_index, W, b):
    in_maps, meta = _preprocess(x, edge_index, W, b)
    res = _run(in_maps, meta, trace=False)
    return _assemble(res.results, meta, N_CORES)


# revision 22
# speedup vs baseline: 1.2476x; 1.0505x over previous
"""GCNConv kernel for Trainium2, SPMD over 8 NeuronCores.

Math (matches the reference):
    row = [edge_index[0], arange(N)]; col = [edge_index[1], arange(N)]
    deg = bincount(row); dis = deg ** -0.5
    agg[c] = sum_{e: col_e == c} dis[row_e] * dis[c] * x[row_e]
    out = agg @ W.T + b

Distribution: edges are sorted by destination column; core k owns
destination columns [k*12500, (k+1)*12500).  Outputs are disjoint across
cores -> no collectives.

Host preprocessing builds, per core, the edge-expanded message stream
x_dis[row_e] (bf16, pre-scaled by the source-side dis factor) laid out in
[128-edge x 128-feat] tiles grouped by destination column block, plus an
exact fp8 0/1 one-hot S mapping each edge slot to its local destination
column.  (The per-edge expansion is done host-side: the batched device
gather primitive -- InstDMAGatherAnt -- corrupts index values >= 256 on
this toolchain (the idx stream is rounded like an 8-bit-mantissa float,
verified empirically), multi-offset indirect DMA ([128,K>=2] offsets)
gathers wrong rows for partitions >= 64, and the working [128,1]-offset
indirect-DMA fallback costs a measured 1.44 us per 128 edges = ~2.7 ms
per core, 8x over the memory roofline.  All verified on hardware.)

Per-core device program (the scatter/segment-sum and the linear):
  - for each group of GRP=4 column blocks (128 destination columns each):
      * stream the group's message tiles + one-hot tiles into SBUF
        (contiguous DMAs on the sync HWDGE queue, triple-buffered)
      * per block: K matmuls  psum[d, c] += G_tile^T @ S_tile  accumulate
        the block's transposed segment sums in PSUM (lhsT = bf16 messages,
        rhs = fp8 one-hot; mixed-dtype matmul)
      * drain PSUM with a fused multiply by dis[col] (fp32 broadcast
        tile, DVE), fp32 matmul with W^T, per-partition bias add (DVE)
      * one batched output store per group on the scalar HWDGE queue
        (keeps the sync queue a pure prefetch stream; interleaving stores
        on the sync queue stalls prefetch and costs ~115 us via PE
        idle + HAM re-throttle)
  - host transposes each core's [128, 12500] slab into the final output.

Measured on HW: 294 us, rel err 1.7e-3 (vs ~291 us DMA-byte floor at
358 GB/s/core for the ~104 MB/core streamed).
"""

import numpy as np
import ml_dtypes

BF16 = ml_dtypes.bfloat16
FP8 = ml_dtypes.float8_e4m3

N_NODES = 100000
D = 128
N_CORES = 8
BLK = 128
GRP = 4  # column blocks per DMA group


def _preprocess(x, edge_index, W, b, n_cores=N_CORES, grp=GRP):
    """Host-side index preprocessing and input sharding."""
    n = x.shape[0]
    d = x.shape[1]
    assert n % n_cores == 0
    cpc = n // n_cores  # destination columns per core
    nblk = -(-cpc // BLK)  # column blocks per core
    pcols = nblk * BLK
    ng = -(-nblk // grp)  # groups per core

    idt = edge_index.dtype
    loop = np.arange(n, dtype=idt)
    row = np.concatenate([np.asarray(edge_index[0]), loop])
    col = np.concatenate([np.asarray(edge_index[1]), loop])

    deg = np.bincount(row, minlength=n)
    dis = (deg.astype(np.float64) ** -0.5).astype(np.float32)

    x_dis = (np.asarray(x) * dis[:, None]).astype(BF16)

    order = np.argsort(col, kind="stable")
    rs = row[order].astype(np.int64)
    cs = col[order].astype(np.int64)

    core = cs // cpc
    lb = (cs - core * cpc) // BLK  # block within core
    cloc = (cs - core * cpc) % BLK  # column within block

    key = core * nblk + lb
    counts = np.bincount(key, minlength=n_cores * nblk)
    starts = np.concatenate([[0], np.cumsum(counts)[:-1]])
    rank = np.arange(len(cs)) - starts[key]
    kmax = int(-(-counts.max() // 128))

    g = lb // grp
    bl = lb % grp
    kt = rank // 128
    p = rank % 128
    w_idx = grp * kmax

    # edge-expanded message stream: [core][g][p][bl*kmax+kt][128 feat]
    xe = np.zeros((n_cores, ng, 128, w_idx, d), BF16)
    flat_tile = ((core * ng + g) * 128 + p) * w_idx + bl * kmax + kt
    xe.reshape(-1, d)[flat_tile] = x_dis[rs]

    s_all = np.zeros((n_cores, ng, 128, w_idx * 128), FP8)
    flat_s = ((core * ng + g) * 128 + p) * (w_idx * 128) + (bl * kmax + kt) * 128 + cloc
    s_all.reshape(-1)[flat_s] = FP8(1.0)

    disb_all = np.zeros((n_cores, 128, pcols), np.float32)
    for k in range(n_cores):
        dc = np.zeros(pcols, np.float32)
        dc[:cpc] = dis[k * cpc : (k + 1) * cpc]
        disb_all[k] = dc[None, :]

    wt = np.ascontiguousarray(np.asarray(W).T.astype(np.float32))
    bias = np.asarray(b).astype(np.float32).reshape(d, 1)

    in_maps = []
    for k in range(n_cores):
        in_maps.append(
            {
                "xe": xe[k].reshape(ng, 128, w_idx * d),
                "soh": s_all[k],
                "wt": wt,
                "bias": bias,
                "disb": disb_all[k],
            }
        )
    meta = dict(n=n, d=d, cpc=cpc, nblk=nblk, pcols=pcols, ng=ng, kmax=kmax, grp=grp)
    return in_maps, meta


def _build_program(meta):
    import concourse.bacc as bacc
    import concourse.tile as tile
    from concourse import mybir

    d = meta["d"]
    ng = meta["ng"]
    grp = meta["grp"]
    kmax = meta["kmax"]
    pcols = meta["pcols"]
    nblk = meta["nblk"]
    w_idx = grp * kmax

    f32 = mybir.dt.float32
    bf16 = mybir.dt.bfloat16
    fp8 = mybir.dt.float8e4

    nc = bacc.Bacc("TRN2", target_bir_lowering=False, debug=False)
    xe_t = nc.declare_dram_parameter("xe", [ng, 128, w_idx * d], bf16, isOutput=False)
    s_t = nc.declare_dram_parameter("soh", [ng, 128, w_idx * 128], fp8, isOutput=False)
    wt_t = nc.declare_dram_parameter("wt", [d, d], f32, isOutput=False)
    b_t = nc.declare_dram_parameter("bias", [d, 1], f32, isOutput=False)
    d_t = nc.declare_dram_parameter("disb", [128, pcols], f32, isOutput=False)
    o_t = nc.declare_dram_parameter("outT", [128, pcols], f32, isOutput=True)

    with tile.TileContext(nc) as tc:
        with (
            tc.tile_pool(name="const", bufs=1) as constp,
            tc.tile_pool(name="gather", bufs=3) as gatherp,
            tc.tile_pool(name="sohp", bufs=3) as sohp,
            tc.tile_pool(name="aggp", bufs=3) as aggp,
            tc.tile_pool(name="outp", bufs=2) as outp,
            tc.tile_pool(name="psA", bufs=4, space="PSUM") as psa,
            tc.tile_pool(name="psB", bufs=2, space="PSUM") as psb,
        ):
            # constants go on the scalar (ACT) HWDGE queue so the sync queue
            # stays a pure xe/soh prefetch stream
            wt_sb = constp.tile([d, d], f32)
            nc.scalar.dma_start(out=wt_sb[:], in_=wt_t[:])
            b_sb = constp.tile([d, 1], f32)
            nc.scalar.dma_start(out=b_sb[:], in_=b_t[:])
            disb_sb = constp.tile([128, pcols], f32)
            nc.scalar.dma_start(out=disb_sb[:], in_=d_t[:])

            for gi in range(ng):
                ar = gatherp.tile([128, w_idx * d], bf16)
                nc.sync.dma_start(out=ar[:], in_=xe_t[gi])
                s_sb = sohp.tile([128, w_idx * 128], fp8)
                nc.sync.dma_start(out=s_sb[:], in_=s_t[gi])

                gblk = min(grp, nblk - gi * grp)  # blocks in this group
                out_g = outp.tile([128, grp * 128], f32, tag="outg")
                for bl in range(gblk):
                    blk = gi * grp + bl
                    ps = psa.tile([128, 128], f32)
                    for kt in range(kmax):
                        off = (bl * kmax + kt) * 128
                        nc.tensor.matmul(
                            out=ps[:],
                            lhsT=ar[:, off : off + 128],
                            rhs=s_sb[:, off : off + 128],
                            start=(kt == 0),
                            stop=(kt == kmax - 1),
                        )
                    agg_sb = aggp.tile([128, 128], f32)
                    nc.vector.tensor_tensor(
                        out=agg_sb[:],
                        in0=ps[:],
                        in1=disb_sb[:, blk * 128 : (blk + 1) * 128],
                        op=mybir.AluOpType.mult,
                    )
                    ps2 = psb.tile([128, 128], f32)
                    nc.tensor.matmul(
                        out=ps2[:], lhsT=wt_sb[:], rhs=agg_sb[:], start=True, stop=True
                    )
                    nc.vector.tensor_scalar_add(
                        out=out_g[:, bl * 128 : (bl + 1) * 128],
                        in0=ps2[:],
                        scalar1=b_sb[:],
                    )
                # one batched store per group, on the scalar HWDGE queue
                nc.scalar.dma_start(
                    out=o_t[:, gi * grp * 128 : gi * grp * 128 + gblk * 128],
                    in_=out_g[:, : gblk * 128],
                )
    nc.compile()
    return nc


def _run(in_maps, meta, trace=False):
    from concourse.bass_utils import run_bass_kernel_spmd

    nc = _build_program(meta)
    n_cores = len(in_maps)
    res = run_bass_kernel_spmd(nc, in_maps, list(range(n_cores)), trace=trace)
    return res


def _assemble(results, meta, n_cores):
    cpc = meta["cpc"]
    out = np.empty((meta["n"], meta["d"]), np.float32)
    for k in range(n_cores):
        out[k * cpc : (k + 1) * cpc, :] = results[k]["outT"][:, :cpc].T
    return out


def kernel(x, edge_index, W, b):
    in_maps, meta = _preprocess(x, edge_index, W, b)
    res = _run(in_maps, meta, trace=False)
    return _assemble(res.results, meta, N_CORES)


# revision 33
# speedup vs baseline: 1.3444x; 1.0776x over previous
"""GCNConv kernel for Trainium2, SPMD over 8 NeuronCores.

Math (matches the reference):
    row = [edge_index[0], arange(N)]; col = [edge_index[1], arange(N)]
    deg = bincount(row); dis = deg ** -0.5
    agg[c] = sum_{e: col_e == c} dis[row_e] * dis[c] * x[row_e]
    out = agg @ W.T + b

Distribution: edges are sorted by destination column; core k owns
destination columns [k*12500, (k+1)*12500).  Outputs are disjoint across
cores -> no collectives.

Host preprocessing builds, per core, the edge-expanded message stream
x_dis[row_e] (bf16, pre-scaled by the source-side dis factor) laid out in
[128-edge x 128-feat] tiles grouped by destination column block, plus an
exact fp8 0/1 one-hot S mapping each edge slot to its local destination
column.  (The per-edge expansion is done host-side: the batched device
gather primitive -- InstDMAGatherAnt -- corrupts index values >= 256 on
this toolchain (the idx stream is rounded like an 8-bit-mantissa float,
verified empirically), multi-offset indirect DMA ([128,K>=2] offsets)
gathers wrong rows for partitions >= 64, and the working [128,1]-offset
indirect-DMA fallback costs a measured 1.44 us per 128 edges = ~2.7 ms
per core, 8x over the memory roofline.  All verified on hardware.)

Per-core device program (the scatter/segment-sum and the linear):
  - for each group of GRP=4 column blocks (128 destination columns each):
      * stream the group's message tiles + one-hot tiles into SBUF
        (contiguous DMAs on the sync HWDGE queue, triple-buffered)
      * per block: K matmuls  psum[d, c] += G_tile^T @ S_tile  accumulate
        the block's transposed segment sums in PSUM (lhsT = bf16 messages,
        rhs = fp8 one-hot; mixed-dtype matmul)
      * drain PSUM with a fused multiply by dis[col] (fp32 broadcast
        tile, DVE), fp32 matmul with W^T, per-partition bias add (DVE)
      * one batched output store per group on the scalar HWDGE queue
        (keeps the sync queue a pure prefetch stream; interleaving stores
        on the sync queue stalls prefetch and costs ~115 us via PE
        idle + HAM re-throttle)
  - host transposes each core's [128, 12500] slab into the final output.

Measured on HW: 294 us, rel err 1.7e-3 (vs ~291 us DMA-byte floor at
358 GB/s/core for the ~104 MB/core streamed).
"""

import numpy as np
import ml_dtypes

BF16 = ml_dtypes.bfloat16
FP8 = ml_dtypes.float8_e4m3

N_NODES = 100000
D = 128
N_CORES = 8
BLK = 128
GRP = 4  # column blocks per DMA group


def _preprocess(x, edge_index, W, b, n_cores=N_CORES, grp=GRP):
    """Host-side index preprocessing and input sharding."""
    n = x.shape[0]
    d = x.shape[1]
    assert n % n_cores == 0
    cpc = n // n_cores  # destination columns per core
    nblk = -(-cpc // BLK)  # column blocks per core
    pcols = nblk * BLK
    ng = -(-nblk // grp)  # groups per core

    idt = edge_index.dtype
    loop = np.arange(n, dtype=idt)
    row = np.concatenate([np.asarray(edge_index[0]), loop])
    col = np.concatenate([np.asarray(edge_index[1]), loop])

    deg = np.bincount(row, minlength=n)
    dis = (deg.astype(np.float64) ** -0.5).astype(np.float32)

    x_dis = (np.asarray(x) * dis[:, None]).astype(BF16)

    order = np.argsort(col, kind="stable")
    rs = row[order].astype(np.int64)
    cs = col[order].astype(np.int64)

    core = cs // cpc
    lb = (cs - core * cpc) // BLK  # block within core
    cloc = (cs - core * cpc) % BLK  # column within block

    key = core * nblk + lb
    counts = np.bincount(key, minlength=n_cores * nblk)
    starts = np.concatenate([[0], np.cumsum(counts)[:-1]])
    rank = np.arange(len(cs)) - starts[key]
    kmax = int(-(-counts.max() // 128))

    g = lb // grp
    bl = lb % grp
    kt = rank // 128
    p = rank % 128
    w_idx = grp * kmax

    # edge-expanded message stream: [core][g][p][bl*kmax+kt][128 feat]
    xe = np.zeros((n_cores, ng, 128, w_idx, d), BF16)
    flat_tile = ((core * ng + g) * 128 + p) * w_idx + bl * kmax + kt
    xe.reshape(-1, d)[flat_tile] = x_dis[rs]

    s_all = np.zeros((n_cores, ng, 128, w_idx * 128), FP8)
    flat_s = ((core * ng + g) * 128 + p) * (w_idx * 128) + (bl * kmax + kt) * 128 + cloc
    s_all.reshape(-1)[flat_s] = FP8(1.0)
    # hybrid one-hot: stream blocks 0..1 of each group, DVE-generate 2..3
    hblk = grp // 2
    soh_half = np.ascontiguousarray(s_all[..., : hblk * kmax * 128])
    clocs = np.full((n_cores, ng, 128, w_idx), 255.0, BF16)
    flat_c = ((core * ng + g) * 128 + p) * w_idx + bl * kmax + kt
    clocs.reshape(-1)[flat_c] = cloc.astype(BF16)
    clocs_half = np.ascontiguousarray(clocs[..., hblk * kmax :])
    iota = np.broadcast_to(np.arange(BLK, dtype=np.float32), (128, BLK)).astype(BF16)

    disb_all = np.zeros((n_cores, 128, pcols), np.float32)
    for k in range(n_cores):
        dc = np.zeros(pcols, np.float32)
        dc[:cpc] = dis[k * cpc : (k + 1) * cpc]
        disb_all[k] = dc[None, :]

    wt = np.ascontiguousarray(np.asarray(W).T.astype(np.float32))
    bias = np.asarray(b).astype(np.float32).reshape(d, 1)

    in_maps = []
    for k in range(n_cores):
        in_maps.append(
            {
                "xe": xe[k].reshape(ng, 128, w_idx * d),
                "soh": soh_half[k],
                "clocs": clocs_half[k],
                "iota": iota,
                "wt": wt,
                "bias": bias,
                "disb": disb_all[k],
            }
        )
    meta = dict(n=n, d=d, cpc=cpc, nblk=nblk, pcols=pcols, ng=ng, kmax=kmax, grp=grp)
    return in_maps, meta


def _build_program(meta):
    import concourse.bacc as bacc
    import concourse.tile as tile
    from concourse import mybir

    d = meta["d"]
    ng = meta["ng"]
    grp = meta["grp"]
    kmax = meta["kmax"]
    pcols = meta["pcols"]
    nblk = meta["nblk"]
    w_idx = grp * kmax

    f32 = mybir.dt.float32
    bf16 = mybir.dt.bfloat16
    fp8 = mybir.dt.float8e4

    hblk = grp // 2
    nc = bacc.Bacc("TRN2", target_bir_lowering=False, debug=False)
    xe_t = nc.declare_dram_parameter("xe", [ng, 128, w_idx * d], bf16, isOutput=False)
    s_t = nc.declare_dram_parameter(
        "soh", [ng, 128, hblk * kmax * 128], fp8, isOutput=False
    )
    c_t = nc.declare_dram_parameter(
        "clocs", [ng, 128, (grp - hblk) * kmax], bf16, isOutput=False
    )
    io_t = nc.declare_dram_parameter("iota", [128, BLK], bf16, isOutput=False)
    wt_t = nc.declare_dram_parameter("wt", [d, d], f32, isOutput=False)
    b_t = nc.declare_dram_parameter("bias", [d, 1], f32, isOutput=False)
    d_t = nc.declare_dram_parameter("disb", [128, pcols], f32, isOutput=False)
    o_t = nc.declare_dram_parameter("outT", [128, pcols], f32, isOutput=True)

    with tile.TileContext(nc) as tc:
        with (
            tc.tile_pool(name="const", bufs=1) as constp,
            tc.tile_pool(name="gather", bufs=3) as gatherp,
            tc.tile_pool(name="sohp", bufs=3) as sohp,
            tc.tile_pool(name="clocp", bufs=3) as clocp,
            tc.tile_pool(name="aggp", bufs=3) as aggp,
            tc.tile_pool(name="outp", bufs=2) as outp,
            tc.tile_pool(name="psA", bufs=4, space="PSUM") as psa,
            tc.tile_pool(name="psB", bufs=2, space="PSUM") as psb,
        ):
            # constants go on the scalar (ACT) HWDGE queue so the sync queue
            # stays a pure xe/soh prefetch stream
            wt_sb = constp.tile([d, d], f32)
            nc.scalar.dma_start(out=wt_sb[:], in_=wt_t[:])
            b_sb = constp.tile([d, 1], f32)
            nc.scalar.dma_start(out=b_sb[:], in_=b_t[:])
            iota_sb = constp.tile([128, BLK], bf16)
            nc.scalar.dma_start(out=iota_sb[:], in_=io_t[:])
            disb_sb = constp.tile([128, pcols], f32)
            nc.scalar.dma_start(out=disb_sb[:], in_=d_t[:])

            for gi in range(ng):
                ar = gatherp.tile([128, w_idx * d], bf16)
                nc.sync.dma_start(out=ar[:], in_=xe_t[gi])
                s_sb = sohp.tile([128, w_idx * 128], fp8)
                nc.sync.dma_start(out=s_sb[:, : hblk * kmax * 128], in_=s_t[gi])
                cloc_sb = clocp.tile([128, (grp - hblk) * kmax], bf16)
                nc.sync.dma_start(out=cloc_sb[:], in_=c_t[gi])
                # DVE-generate the one-hot for the back half of the group:
                # S[p, t, c] = (cloc[p, t] == c), fp8 0/1 exact
                for h in range(grp - hblk):
                    t0 = (hblk + h) * kmax
                    nc.vector.tensor_tensor(
                        out=s_sb[:, t0 * 128 : (t0 + kmax) * 128].rearrange(
                            "p (t c) -> p t c", c=128
                        ),
                        in0=iota_sb[:]
                        .rearrange("p (a c) -> p a c", a=1)
                        .broadcast_to([128, kmax, 128]),
                        in1=cloc_sb[:, h * kmax : (h + 1) * kmax]
                        .rearrange("p (t a) -> p t a", a=1)
                        .broadcast_to([128, kmax, 128]),
                        op=mybir.AluOpType.is_equal,
                    )

                gblk = min(grp, nblk - gi * grp)  # blocks in this group
                out_g = outp.tile([128, grp * 128], f32, tag="outg")
                for bl in range(gblk):
                    blk = gi * grp + bl
                    ps = psa.tile([128, 128], f32)
                    for kt in range(kmax):
                        off = (bl * kmax + kt) * 128
                        nc.tensor.matmul(
                            out=ps[:],
                            lhsT=ar[:, off : off + 128],
                            rhs=s_sb[:, off : off + 128],
                            start=(kt == 0),
                            stop=(kt == kmax - 1),
                        )
                    agg_sb = aggp.tile([128, 128], f32)
                    nc.vector.tensor_tensor(
                        out=agg_sb[:],
                        in0=ps[:],
                        in1=disb_sb[:, blk * 128 : (blk + 1) * 128],
                        op=mybir.AluOpType.mult,
                    )
                    ps2 = psb.tile([128, 128], f32)
                    nc.tensor.matmul(
                        out=ps2[:], lhsT=wt_sb[:], rhs=agg_sb[:], start=True, stop=True
                    )
                    nc.vector.tensor_scalar_add(
                        out=out_g[:, bl * 128 : (bl + 1) * 128],
                        in0=ps2[:],
                        scalar1=b_sb[:],
                    )
                # one batched store per group, on the scalar HWDGE queue
                nc.scalar.dma_start(
                    out=o_t[:, gi * grp * 128 : gi * grp * 128 + gblk * 128],
                    in_=out_g[:, : gblk * 128],
                )
    nc.compile()
    return nc


def _run(in_maps, meta, trace=False):
    from concourse.bass_utils import run_bass_kernel_spmd

    nc = _build_program(meta)
    n_cores = len(in_maps)
    res = run_bass_kernel_spmd(nc, in_maps, list(range(n_cores)), trace=trace)
    return res


def _assemble(results, meta, n_cores):
    cpc = meta["cpc"]
    out = np.empty((meta["n"], meta["d"]), np.float32)
    for k in range(n_cores):
        out[k * cpc : (k + 1) * cpc, :] = results[k]["outT"][:, :cpc].T
    return out


def kernel(x, edge_index, W, b):
    in_maps, meta = _preprocess(x, edge_index, W, b)
    res = _run(in_maps, meta, trace=False)
    return _assemble(res.results, meta, N_CORES)
